# revision 20
# baseline (speedup 1.0000x reference)
"""CompressedLinear trn2 kernel (v2).

Computes y = x @ (Q * scales).T + (x @ D.T) @ U.T   for
x [8192, 4096] fp32, Q [4096, 4096] int32 (values 0..126),
scales [4096, 1] fp32, U [4096, 64] fp32, D [64, 4096] fp32.

Strategy: token-parallel over 8 NeuronCores (each core owns 1024 tokens and
computes its full output rows locally; no collectives). Each core:
  - keeps its x.T slice resident in SBUF as bf16 (8 MiB),
  - streams Q.T in 1 MiB bf16 o-panel slabs (values < 127 are exact in bf16),
  - computes y.T tiles [128 o, 512 n] on the PE with bf16 matmuls into fp32
    PSUM (x bf16 rounding gives ~1.8e-3 scale-relative error, well inside the
    2e-2 gate),
  - the low-rank adapter is folded into the SAME PSUM accumulation: host
    pre-divides U by scales, so psum = x@Q.T + (x@D.T)@(U/s).T and the single
    ACT epilogue multiply by scales[o] produces the final output — no DVE add,
    no separate adapter banks,
  - Q-slab DMAs are issued ahead of the bulk x stream so the PE starts within
    a few microseconds; the first two o-panels are interleaved i-tile-wise so
    the PE chases the incoming x stream without stalling,
  - writes y.T [4096, 1024] to DRAM; host reassembles y.

All host-side work is layout/cast only (plus the U/scales reparametrization
of the adapter); every FLOP of the operator runs on device.
"""

import numpy as np

import concourse.mybir as mybir
import concourse.tile as tile
from concourse import bacc
from concourse import bass_utils as _bass_utils
from concourse.bass_utils import run_bass_kernel_spmd

# walrus ldw-opt (elide duplicate LDWEIGHTS) rejects bf16 stationary tiles
# ("InstLdweights is not compatible with LDW optimization"); the trace shows
# LDWEIGHTS fully pipelined behind matmuls anyway, so leave it off.
LDW_OPT = False

_orig_run_command = _bass_utils.run_command


def _patched_run_command(argv, **kwargs):
    if LDW_OPT:
        argv = [
            a.replace("--enable-ldw-opt=false", "--enable-ldw-opt=true")
            if isinstance(a, str) else a
            for a in argv
        ]
    return _orig_run_command(argv, **kwargs)


_bass_utils.run_command = _patched_run_command

N_TOKENS = 8192
D_IN = 4096
D_OUT = 4096
RANK = 64
N_CORES = 8
N_TOK = N_TOKENS // N_CORES      # 1024 tokens per core
NBLK = 512                       # moving free dim per matmul (PSUM bank)
NB = N_TOK // NBLK               # 2 n-blocks
NI = D_IN // 128                 # 32 contraction tiles
NO = D_OUT // 128                # 32 output-dim tiles
NHEAD = 3                        # o-panels interleaved while x streams in
LOOKAHEAD = 4                    # q-slab prefetch depth in the steady loop
BF16 = mybir.dt.bfloat16
F32 = mybir.dt.float32

_cached_nc = None


def _build():
    nc = bacc.Bacc(None, target_bir_lowering=False)

    # DRAM I/O (per core), all bf16 except scales/output.
    xT = nc.dram_tensor("xT", [128, NI * N_TOK], BF16, kind="ExternalInput")
    q6 = nc.dram_tensor("q6", [NO, 128, NI * 128], BF16, kind="ExternalInput")
    dT = nc.dram_tensor("dT", [128, NI * RANK], BF16, kind="ExternalInput")
    # uT holds U/scales (adapter folded into the pre-scale accumulation)
    uT = nc.dram_tensor("uT", [NO, RANK, 128], BF16, kind="ExternalInput")
    sc = nc.dram_tensor("sc", [128, NO], F32, kind="ExternalInput")
    yT = nc.dram_tensor("yT", [D_OUT, N_TOK], F32, kind="ExternalOutput")

    with tile.TileContext(nc) as tc:
        with (
            tc.tile_pool(name="xp", bufs=1) as xpool,
            tc.tile_pool(name="qp", bufs=6) as qpool,
            tc.tile_pool(name="dp", bufs=1) as dpool,
            tc.tile_pool(name="up", bufs=6) as upool,
            tc.tile_pool(name="sp", bufs=1) as spool,
            tc.tile_pool(name="tp", bufs=1) as tpool,
            tc.tile_pool(name="op", bufs=2) as opool,
            tc.tile_pool(name="pm", bufs=6, space="PSUM") as psm,
            tc.tile_pool(name="pa", bufs=2, space="PSUM") as psa,
        ):
            # dT leads the sync queue: the PE warmup depends on its first
            # chunk, and the gpsimd (SWDGE) queue spins up noticeably later.
            dT_sb = dpool.tile([128, NI * RANK], BF16)
            nc.sync.dma_start(dT_sb[:, 0:NBLK], dT[:, 0:NBLK])
            nc.sync.dma_start(dT_sb[:, NBLK:], dT[:, NBLK:])

            xT_sb = xpool.tile([128, NI * N_TOK], BF16)

            def fetch_x(lo, hi):
                nc.sync.dma_start(
                    xT_sb[:, lo * N_TOK:hi * N_TOK], xT[:, lo * N_TOK:hi * N_TOK]
                )

            qs, us = {}, {}

            def fetch(ot, span=None):
                if ot not in qs:
                    qs[ot] = qpool.tile([128, NI * 128], BF16, name="qslab")
                lo, hi = span or (0, NI * 128)
                nc.sync.dma_start(qs[ot][:, lo:hi], q6[ot, :, lo:hi])

            def fetch_u(ot):
                u_sb = upool.tile([RANK, 128], BF16, name="uslab")
                nc.gpsimd.dma_start(u_sb[:], uT[ot])
                us[ot] = u_sb

            # DMA priority order: the PE's first dependencies stream first in
            # fine chunks — x i-tile pairs round-robined with q0-q2 quarter
            # slabs so the interleaved 3-panel head never starves.
            QC = NI * 128 // 4       # quarter slab
            fetch_x(0, 1)
            fetch(0, (0, QC))
            fetch(1, (0, QC))
            fetch_x(1, 2)
            fetch(2, (0, QC))
            fetch_u(0)
            fetch_u(1)
            fetch_u(2)
            for c in range(1, 4):
                fetch_x(2 * c, 2 * c + 2)
                fetch(0, (c * QC, (c + 1) * QC))
                fetch(1, (c * QC, (c + 1) * QC))
                fetch(2, (c * QC, (c + 1) * QC))
            sc_sb = spool.tile([128, NO], F32)
            nc.gpsimd.dma_start(sc_sb[:], sc[:])
            fetch_x(8, 12)
            fetch(3)
            fetch_u(3)
            fetch_x(12, 20)
            fetch_x(20, 32)

            tT_sb = tpool.tile([RANK, N_TOK], BF16)
            pms = {}

            # p-state warmup: keep the PE busy on dT while x/q stream in so
            # the clock is ramped when the real matmuls start. The scratch
            # accumulator is never read; each matmul is its own group.
            warm = psa.tile([RANK, NBLK], F32, name="pat")
            for _ in range(5):
                nc.tensor.matmul(
                    warm[:], dT_sb[:, 0:RANK], dT_sb[:, 0:NBLK],
                    start=True, stop=True, skip_group_check=True,
                )

            def emit_mms(ot, it):
                q_sb = qs[ot]
                for nb in range(NB):
                    nc.tensor.matmul(
                        pms[ot][nb][:],
                        q_sb[:, it * 128:(it + 1) * 128],
                        xT_sb[:, it * N_TOK + nb * NBLK:
                              it * N_TOK + nb * NBLK + NBLK],
                        start=(it == 0),
                        stop=False,
                    )

            def emit_tail(ot):
                # adapter folds into the open main accumulation, then a single
                # ACT pass applies scales[o] to (main + adapt/s).
                u_sb = us.pop(ot)
                o_sb = opool.tile([128, N_TOK], F32, name="ostage")
                for nb in range(NB):
                    nc.tensor.matmul(
                        pms[ot][nb][:],
                        u_sb[:],
                        tT_sb[:, nb * NBLK:(nb + 1) * NBLK],
                        start=False,
                        stop=True,
                    )
                    nc.scalar.mul(
                        o_sb[:, nb * NBLK:(nb + 1) * NBLK],
                        pms[ot][nb][:],
                        sc_sb[:, ot:ot + 1],
                    )
                del pms[ot]
                qs.pop(ot)
                nc.sync.dma_start(yT[ot * 128:(ot + 1) * 128, :], o_sb[:])

            # Head: NHEAD panels interleaved i-tile-wise, chasing the x stream.
            for ot in range(NHEAD):
                pms[ot] = [
                    psm.tile([128, NBLK], F32, name="pmt") for _ in range(NB)
                ]
            for it in range(NI):
                for ot in range(NHEAD):
                    emit_mms(ot, it)

            # t.T = D @ x.T  [64, N_TOK] (needs all of x), kept resident bf16
            for nb in range(NB):
                pt = psa.tile([RANK, NBLK], F32, name="pat")
                for it in range(NI):
                    nc.tensor.matmul(
                        pt[:],
                        dT_sb[:, it * RANK:(it + 1) * RANK],
                        xT_sb[:, it * N_TOK + nb * NBLK:
                              it * N_TOK + nb * NBLK + NBLK],
                        start=(it == 0),
                        stop=(it == NI - 1),
                    )
                nc.vector.tensor_copy(
                    tT_sb[:, nb * NBLK:(nb + 1) * NBLK], pt[:]
                )

            for ot in range(NHEAD):
                emit_tail(ot)
            for ot in range(NHEAD, NO):
                for nxt in range(ot, min(ot + LOOKAHEAD, NO)):
                    if nxt not in qs:
                        fetch(nxt)
                        fetch_u(nxt)
                pms[ot] = [
                    psm.tile([128, NBLK], F32, name="pmt") for _ in range(NB)
                ]
                for it in range(NI):
                    emit_mms(ot, it)
                emit_tail(ot)

    nc.compile()
    return nc


def kernel(x, scales, U, D, Q, _trace=False, _trace_cores=None):
    global _cached_nc
    if _cached_nc is None:
        _cached_nc = _build()
    nc = _cached_nc

    import ml_dtypes
    bf = ml_dtypes.bfloat16

    x = np.asarray(x, dtype=np.float32)
    scales = np.asarray(scales, dtype=np.float32)
    U = np.asarray(U, dtype=np.float32)
    D = np.asarray(D, dtype=np.float32)
    Q = np.asarray(Q)

    # Host layout prep (pure permutation/cast, plus the U/scales fold):
    # x7[c, p, it, n] = x[c*N_TOK + n, it*128 + p]
    x7 = np.ascontiguousarray(
        x.reshape(N_CORES, N_TOK, NI, 128).transpose(0, 3, 2, 1).astype(bf)
    ).reshape(N_CORES, 128, NI * N_TOK)
    # q6[ot, p, it, oc] = Q[ot*128 + oc, it*128 + p]; ints < 127 are exact
    # in bf16 (8-bit mantissa)
    q6 = np.ascontiguousarray(
        Q.reshape(NO, 128, NI, 128).transpose(0, 3, 2, 1).astype(bf)
    ).reshape(NO, 128, NI * 128)
    # dT7[p, it, r] = D[r, it*128 + p]
    dT7 = np.ascontiguousarray(
        D.reshape(RANK, NI, 128).transpose(2, 1, 0).astype(bf)
    ).reshape(128, NI * RANK)
    # uT8[ot, r, oc] = (U/scales)[ot*128 + oc, r] — adapter pre-divided by
    # scales so it can accumulate into the pre-scale PSUM
    Up = (U / scales).astype(np.float32)
    uT8 = np.ascontiguousarray(
        Up.reshape(NO, 128, RANK).transpose(0, 2, 1).astype(bf)
    )
    # sc7[p, ot] = scales[ot*128 + p]
    sc7 = np.ascontiguousarray(scales.reshape(NO, 128).T)

    in_maps = [
        {"xT": x7[c], "q6": q6, "dT": dT7, "uT": uT8, "sc": sc7}
        for c in range(N_CORES)
    ]
    kwargs = {}
    if _trace:
        kwargs["trace"] = True
        kwargs["trace_cores"] = _trace_cores or [0]
    r = run_bass_kernel_spmd(nc, in_maps, core_ids=list(range(N_CORES)), **kwargs)
    kernel.last_results = r

    y = np.empty((N_TOKENS, D_OUT), dtype=np.float32)
    for c in range(N_CORES):
        y[c * N_TOK:(c + 1) * N_TOK, :] = r.results[c]["yT"].T
    return y


# revision 21
# speedup vs baseline: 1.0130x; 1.0130x over previous
"""CompressedLinear trn2 kernel (v2).

Computes y = x @ (Q * scales).T + (x @ D.T) @ U.T   for
x [8192, 4096] fp32, Q [4096, 4096] int32 (values 0..126),
scales [4096, 1] fp32, U [4096, 64] fp32, D [64, 4096] fp32.

Strategy: token-parallel over 8 NeuronCores (each core owns 1024 tokens and
computes its full output rows locally; no collectives). Each core:
  - keeps its x.T slice resident in SBUF as bf16 (8 MiB),
  - streams Q.T in 1 MiB bf16 o-panel slabs (values < 127 are exact in bf16),
  - computes y.T tiles [128 o, 512 n] on the PE with bf16 matmuls into fp32
    PSUM (x bf16 rounding gives ~1.8e-3 scale-relative error, well inside the
    2e-2 gate),
  - the low-rank adapter is folded into the SAME PSUM accumulation: host
    pre-divides U by scales, so psum = x@Q.T + (x@D.T)@(U/s).T and the single
    ACT epilogue multiply by scales[o] produces the final output — no DVE add,
    no separate adapter banks,
  - Q-slab DMAs are issued ahead of the bulk x stream so the PE starts within
    a few microseconds; the first two o-panels are interleaved i-tile-wise so
    the PE chases the incoming x stream without stalling,
  - writes y.T [4096, 1024] to DRAM; host reassembles y.

All host-side work is layout/cast only (plus the U/scales reparametrization
of the adapter); every FLOP of the operator runs on device.
"""

import numpy as np

import concourse.mybir as mybir
import concourse.tile as tile
from concourse import bacc
from concourse import bass_utils as _bass_utils
from concourse.bass_utils import run_bass_kernel_spmd

# walrus ldw-opt (elide duplicate LDWEIGHTS) rejects bf16 stationary tiles
# ("InstLdweights is not compatible with LDW optimization"); the trace shows
# LDWEIGHTS fully pipelined behind matmuls anyway, so leave it off.
LDW_OPT = False

_orig_run_command = _bass_utils.run_command


def _patched_run_command(argv, **kwargs):
    if LDW_OPT:
        argv = [
            a.replace("--enable-ldw-opt=false", "--enable-ldw-opt=true")
            if isinstance(a, str) else a
            for a in argv
        ]
    return _orig_run_command(argv, **kwargs)


_bass_utils.run_command = _patched_run_command

N_TOKENS = 8192
D_IN = 4096
D_OUT = 4096
RANK = 64
N_CORES = 8
N_TOK = N_TOKENS // N_CORES      # 1024 tokens per core
NBLK = 512                       # moving free dim per matmul (PSUM bank)
NB = N_TOK // NBLK               # 2 n-blocks
NI = D_IN // 128                 # 32 contraction tiles
NO = D_OUT // 128                # 32 output-dim tiles
NHEAD = 3                        # o-panels interleaved while x streams in
LOOKAHEAD = 4                    # q-slab prefetch depth in the steady loop
BF16 = mybir.dt.bfloat16
F32 = mybir.dt.float32

_cached_nc = None


def _build():
    nc = bacc.Bacc(None, target_bir_lowering=False)

    # DRAM I/O (per core), all bf16 except scales/output.
    xT = nc.dram_tensor("xT", [128, NI * N_TOK], BF16, kind="ExternalInput")
    q6 = nc.dram_tensor("q6", [NO, 128, NI * 128], BF16, kind="ExternalInput")
    dT = nc.dram_tensor("dT", [128, NI * RANK], BF16, kind="ExternalInput")
    # uT holds U/scales (adapter folded into the pre-scale accumulation)
    uT = nc.dram_tensor("uT", [NO, RANK, 128], BF16, kind="ExternalInput")
    sc = nc.dram_tensor("sc", [128, NO], F32, kind="ExternalInput")
    yT = nc.dram_tensor("yT", [D_OUT, N_TOK], F32, kind="ExternalOutput")

    with tile.TileContext(nc) as tc:
        with (
            tc.tile_pool(name="xp", bufs=1) as xpool,
            tc.tile_pool(name="qp", bufs=6) as qpool,
            tc.tile_pool(name="dp", bufs=1) as dpool,
            tc.tile_pool(name="up", bufs=6) as upool,
            tc.tile_pool(name="sp", bufs=1) as spool,
            tc.tile_pool(name="tp", bufs=1) as tpool,
            tc.tile_pool(name="op", bufs=2) as opool,
            tc.tile_pool(name="pm", bufs=6, space="PSUM") as psm,
            tc.tile_pool(name="pa", bufs=2, space="PSUM") as psa,
        ):
            # dT leads the sync queue: the PE warmup depends on its first
            # chunk, and the gpsimd (SWDGE) queue spins up noticeably later.
            dT_sb = dpool.tile([128, NI * RANK], BF16)
            nc.sync.dma_start(dT_sb[:, 0:NBLK], dT[:, 0:NBLK])
            nc.sync.dma_start(dT_sb[:, NBLK:], dT[:, NBLK:])

            xT_sb = xpool.tile([128, NI * N_TOK], BF16)

            def fetch_x(lo, hi):
                nc.sync.dma_start(
                    xT_sb[:, lo * N_TOK:hi * N_TOK], xT[:, lo * N_TOK:hi * N_TOK]
                )

            qs, us = {}, {}

            def fetch(ot, span=None):
                if ot not in qs:
                    qs[ot] = qpool.tile([128, NI * 128], BF16, name="qslab")
                lo, hi = span or (0, NI * 128)
                nc.sync.dma_start(qs[ot][:, lo:hi], q6[ot, :, lo:hi])

            def fetch_u(ot):
                u_sb = upool.tile([RANK, 128], BF16, name="uslab")
                nc.gpsimd.dma_start(u_sb[:], uT[ot])
                us[ot] = u_sb

            # DMA priority order: the PE's first dependencies stream first in
            # fine chunks — x i-tile pairs round-robined with q0-q2 quarter
            # slabs so the interleaved 3-panel head never starves.
            QC = NI * 128 // 4       # quarter slab
            fetch_x(0, 1)
            fetch(0, (0, QC))
            fetch(1, (0, QC))
            fetch_x(1, 2)
            fetch(2, (0, QC))
            fetch_u(0)
            fetch_u(1)
            fetch_u(2)
            for c in range(1, 4):
                fetch_x(2 * c, 2 * c + 2)
                fetch(0, (c * QC, (c + 1) * QC))
                fetch(1, (c * QC, (c + 1) * QC))
                fetch(2, (c * QC, (c + 1) * QC))
            sc_sb = spool.tile([128, NO], F32)
            nc.gpsimd.dma_start(sc_sb[:], sc[:])
            fetch_x(8, 12)
            fetch(3)
            fetch_u(3)
            fetch_x(12, 20)
            fetch_x(20, 32)

            tT_sb = tpool.tile([RANK, N_TOK], BF16)
            pms = {}

            # p-state warmup: keep the PE busy on dT while x/q stream in so
            # the clock is ramped when the real matmuls start. The scratch
            # accumulator is never read; each matmul is its own group.
            warm = psa.tile([RANK, NBLK], F32, name="pat")
            for _ in range(12):
                nc.tensor.matmul(
                    warm[:], dT_sb[:, 0:RANK], dT_sb[:, 0:NBLK],
                    start=True, stop=True, skip_group_check=True,
                )

            def emit_mms(ot, it):
                q_sb = qs[ot]
                for nb in range(NB):
                    nc.tensor.matmul(
                        pms[ot][nb][:],
                        q_sb[:, it * 128:(it + 1) * 128],
                        xT_sb[:, it * N_TOK + nb * NBLK:
                              it * N_TOK + nb * NBLK + NBLK],
                        start=(it == 0),
                        stop=False,
                    )

            def emit_tail(ot):
                # adapter folds into the open main accumulation, then a single
                # ACT pass applies scales[o] to (main + adapt/s).
                u_sb = us.pop(ot)
                o_sb = opool.tile([128, N_TOK], F32, name="ostage")
                for nb in range(NB):
                    nc.tensor.matmul(
                        pms[ot][nb][:],
                        u_sb[:],
                        tT_sb[:, nb * NBLK:(nb + 1) * NBLK],
                        start=False,
                        stop=True,
                    )
                    nc.scalar.mul(
                        o_sb[:, nb * NBLK:(nb + 1) * NBLK],
                        pms[ot][nb][:],
                        sc_sb[:, ot:ot + 1],
                    )
                del pms[ot]
                qs.pop(ot)
                nc.sync.dma_start(yT[ot * 128:(ot + 1) * 128, :], o_sb[:])

            # Head: NHEAD panels interleaved i-tile-wise, chasing the x stream.
            for ot in range(NHEAD):
                pms[ot] = [
                    psm.tile([128, NBLK], F32, name="pmt") for _ in range(NB)
                ]
            for it in range(NI):
                for ot in range(NHEAD):
                    emit_mms(ot, it)

            # t.T = D @ x.T  [64, N_TOK] (needs all of x), kept resident bf16
            for nb in range(NB):
                pt = psa.tile([RANK, NBLK], F32, name="pat")
                for it in range(NI):
                    nc.tensor.matmul(
                        pt[:],
                        dT_sb[:, it * RANK:(it + 1) * RANK],
                        xT_sb[:, it * N_TOK + nb * NBLK:
                              it * N_TOK + nb * NBLK + NBLK],
                        start=(it == 0),
                        stop=(it == NI - 1),
                    )
                nc.vector.tensor_copy(
                    tT_sb[:, nb * NBLK:(nb + 1) * NBLK], pt[:]
                )

            for ot in range(NHEAD):
                emit_tail(ot)
            for ot in range(NHEAD, NO):
                for nxt in range(ot, min(ot + LOOKAHEAD, NO)):
                    if nxt not in qs:
                        fetch(nxt)
                        fetch_u(nxt)
                pms[ot] = [
                    psm.tile([128, NBLK], F32, name="pmt") for _ in range(NB)
                ]
                for it in range(NI):
                    emit_mms(ot, it)
                emit_tail(ot)

    nc.compile()
    return nc


def kernel(x, scales, U, D, Q, _trace=False, _trace_cores=None):
    global _cached_nc
    if _cached_nc is None:
        _cached_nc = _build()
    nc = _cached_nc

    import ml_dtypes
    bf = ml_dtypes.bfloat16

    x = np.asarray(x, dtype=np.float32)
    scales = np.asarray(scales, dtype=np.float32)
    U = np.asarray(U, dtype=np.float32)
    D = np.asarray(D, dtype=np.float32)
    Q = np.asarray(Q)

    # Host layout prep (pure permutation/cast, plus the U/scales fold):
    # x7[c, p, it, n] = x[c*N_TOK + n, it*128 + p]
    x7 = np.ascontiguousarray(
        x.reshape(N_CORES, N_TOK, NI, 128).transpose(0, 3, 2, 1).astype(bf)
    ).reshape(N_CORES, 128, NI * N_TOK)
    # q6[ot, p, it, oc] = Q[ot*128 + oc, it*128 + p]; ints < 127 are exact
    # in bf16 (8-bit mantissa)
    q6 = np.ascontiguousarray(
        Q.reshape(NO, 128, NI, 128).transpose(0, 3, 2, 1).astype(bf)
    ).reshape(NO, 128, NI * 128)
    # dT7[p, it, r] = D[r, it*128 + p]
    dT7 = np.ascontiguousarray(
        D.reshape(RANK, NI, 128).transpose(2, 1, 0).astype(bf)
    ).reshape(128, NI * RANK)
    # uT8[ot, r, oc] = (U/scales)[ot*128 + oc, r] — adapter pre-divided by
    # scales so it can accumulate into the pre-scale PSUM
    Up = (U / scales).astype(np.float32)
    uT8 = np.ascontiguousarray(
        Up.reshape(NO, 128, RANK).transpose(0, 2, 1).astype(bf)
    )
    # sc7[p, ot] = scales[ot*128 + p]
    sc7 = np.ascontiguousarray(scales.reshape(NO, 128).T)

    in_maps = [
        {"xT": x7[c], "q6": q6, "dT": dT7, "uT": uT8, "sc": sc7}
        for c in range(N_CORES)
    ]
    kwargs = {}
    if _trace:
        kwargs["trace"] = True
        kwargs["trace_cores"] = _trace_cores or [0]
    r = run_bass_kernel_spmd(nc, in_maps, core_ids=list(range(N_CORES)), **kwargs)
    kernel.last_results = r

    y = np.empty((N_TOKENS, D_OUT), dtype=np.float32)
    for c in range(N_CORES):
        y[c * N_TOK:(c + 1) * N_TOK, :] = r.results[c]["yT"].T
    return y
